# revision 13
# baseline (speedup 1.0000x reference)
"""Trainium2 Bass kernel for nn_DUF_Wguide_resolutionRGB (dense_cnn).

Sharding: 8 cores = 4 dense blocks x 2 batch halves.  Core c runs block
c//2 on samples {0,1} (c even) or {2,3} (c odd).  BatchNorm statistics are
combined across each pair of cores with a tiny AllReduce.

Per-core layout: activations are stored [C, n, H(+2), WD] in DRAM with the
channel dim on SBUF partitions.  Convs are tap-accumulated PE matmuls into
PSUM over 512-position tiles (2 rows x 256 cols).  3x3 WD-edges are handled
by clipping matmul column ranges; H-edges by zero pad rows.

relu(bn(v)) is computed as rsd * relu(v - mean): the sub+max is one DVE
tensor_scalar op, and rsd is folded into the consumer conv's weights.
"""

import contextlib
import ctypes
import sys
import types

import numpy as np

try:
    import concourse.bass as bass
except ImportError:  # pragma: no cover
    sys.path.insert(0, "/opt/trn_rl_repo")
    import concourse.bass as bass

import concourse.tile as tile
from concourse import mybir
from concourse.vector_clock import ScopedClock, VectorClock

F32 = mybir.dt.float32
ADD = mybir.AluOpType.add
MULT = mybir.AluOpType.mult
SUB = mybir.AluOpType.subtract
MAX = mybir.AluOpType.max
AX = mybir.ActivationFunctionType
AXL = mybir.AxisListType

H = WD = 256
NSAMP = 2          # samples per core
RM = 8             # rows per macro tile
NMACRO = H // RM   # 32
NSUB = RM // 2     # psum sub tiles (512 positions each) per macro
NTOT = 4 * H * WD   # global batchnorm count (full batch of 4)
EPS = 1e-5
PAIRS = [[0, 1], [2, 3], [4, 5], [6, 7]]
TAPS9 = [(dy, dx) for dy in (-1, 0, 1) for dx in (-1, 0, 1)]
# dx==0 taps first so the first (start=True) matmul covers the full psum tile
TAPS9_MM = sorted(TAPS9, key=lambda t: (t[1] != 0,))

PROFILE = False
LAST_EXEC_NS = None

_PROG = None


def _install_tile_drain_patch():
    """walrus in this container rejects multi-wait Drain instructions
    ("Too many sync wait commands").  Split the final global-clock waits
    over single-wait SP nops; the SP FIFO then makes the drain safe."""

    def _drain_and_barrier(self, tick_clock, wait_clock):
        nc = self.nc
        vc = tick_clock.global_clock
        n = len(vc)
        for p in range(n):
            t = vc[p]
            if t > 0:
                sub = VectorClock([t if i == p else 0 for i in range(n)])
                nop = nc.sync.nop(nofuse=True, hint=f"predrain_wait_p{p}")
                wait_clock.add_sem_waits(nop.ins, ScopedClock({None: sub}))
        nc.sync.drain()
        nc.all_engine_barrier()
        assert self.sems is not None
        popped = nc._tile_sem_poison_stack.pop()
        assert popped is self._sem_poison
        nc.clear_and_free_semaphores(list(self.sems.allocated().values()))
        nc.all_engine_barrier()

    tile.TileContext._drain_and_barrier = _drain_and_barrier


def _install_ntff_hook():
    """antenv.axon_hooks is missing in this image; inject an equivalent so
    run_bass_kernel_spmd(trace=True) can capture NTFF profiles."""
    if "antenv.axon_hooks" in sys.modules:
        return
    so_path = "/opt/axon/libaxon_pjrt.so"
    try:
        lib = ctypes.CDLL(so_path)
    except OSError:
        return
    if not hasattr(lib, "axon_start_nrt_profile"):
        return
    lib.axon_start_nrt_profile.argtypes = [ctypes.POINTER(ctypes.c_int64), ctypes.c_size_t]
    lib.axon_start_nrt_profile.restype = ctypes.c_int64
    lib.axon_stop_nrt_profile.argtypes = [ctypes.c_char_p]
    lib.axon_stop_nrt_profile.restype = ctypes.c_int64

    @contextlib.contextmanager
    def _hook(output_dir, device_ids):
        import jax

        jax.devices()
        if device_ids:
            ids = (ctypes.c_int64 * len(device_ids))(*device_ids)
            rc = lib.axon_start_nrt_profile(ids, len(device_ids))
        else:
            rc = lib.axon_start_nrt_profile(None, 0)
        if rc != 0:
            raise RuntimeError(f"axon_start_nrt_profile rc={rc}")
        try:
            yield
        finally:
            n = lib.axon_stop_nrt_profile(str(output_dir).encode())
            print(f"ntff profile: {n} file(s) -> {output_dir}", file=sys.stderr)

    mod = types.ModuleType("antenv.axon_hooks")
    mod.get_axon_ntff_profile_hook = lambda: _hook
    mod.set_axon_ntff_profile_hook = lambda h: None
    sys.modules["antenv.axon_hooks"] = mod


# ---------------------------------------------------------------------------
# device program
# ---------------------------------------------------------------------------


def _split_multi_waits(nc):
    """walrus here accepts at most one sync-wait per instruction.  Move every
    wait of a multi-wait instruction onto single-wait NoOps inserted just
    before it on the same engine (the engine sequencer evaluates them in FIFO
    order, which is the semantics Tile assumed for compute engines; for DMA it
    conservatively moves the wait from the DGE queue to the issuing
    sequencer)."""
    cnt = 0
    for bb in nc.main_func.blocks:
        out = []
        for ins in bb.instructions:
            si = ins.sync_info
            if si is not None and si.on_wait and len(si.on_wait) > 1:
                for wsp in si.on_wait:
                    cnt += 1
                    out.append(mybir.InstNoOp(
                        name=f"I-wsplit-{cnt}", opcode="NoOp", engine=ins.engine,
                        sync_info=mybir.SyncInfo(on_wait=[wsp], on_update=[]),
                        text_hint="waitsplit"))
                si.on_wait = []
            out.append(ins)
        bb.instructions = out
    return cnt


def _build_program():
    _install_tile_drain_patch()
    nc = bass.Bass(num_devices=8)

    wg_t = nc.declare_dram_parameter("wg", [4, NSAMP, H + 2, WD + 2], F32, isOutput=False)
    img_t = nc.declare_dram_parameter("img", [NSAMP, H + 4, WD + 4], F32, isOutput=False)
    out_t = nc.declare_dram_parameter("out", [NSAMP, 2, H, 2, WD], F32, isOutput=True)

    wdefs = {
        "pre_w": [36, 64], "pre_b": [64, 1],
        "d1a_w": [64, 64], "d1a_b": [64, 1],
        "d1b_w": [64, 9, 32], "d1b_b": [32, 1],
        "d2a_w": [96, 96], "d2a_b": [96, 1],
        "d2b_w": [96, 9, 32], "d2b_b": [32, 1],
        "main_w": [128, 9, 128], "main_b": [128, 1],
        "r1_w": [128, 128], "r1_b": [128, 1],
        "r2_w": [128, 4], "r2_b": [4, 1],
        "f1_w": [128, 256], "f1_b": [128, 2],
        "f2_w": [128, 2, 100], "f2_b": [100, 1],
        "bmat": [100, 4], "rmat": [25, 100],
    }
    wparams = {k: nc.declare_dram_parameter(k, s, F32, isOutput=False) for k, s in wdefs.items()}

    with tile.TileContext(nc) as tc, contextlib.ExitStack() as ctx:
        consts = ctx.enter_context(tc.tile_pool(name="consts", bufs=1))
        stats = ctx.enter_context(tc.tile_pool(name="stats", bufs=1))
        dram = ctx.enter_context(tc.tile_pool(name="dram", bufs=1, space="DRAM"))

        w = {}
        for k, s in wdefs.items():
            w[k] = consts.tile(s, F32, tag=f"w_{k}", name=f"w_{k}")
            nc.sync.dma_start(out=w[k], in_=wparams[k][:])

        # DRAM intermediates ([C, n, H+2, WD] row-padded except y)
        x_pad = dram.tile([64, NSAMP, H + 2, WD], F32, tag="x_pad")
        t_pad = dram.tile([64, NSAMP, H + 2, WD], F32, tag="t_pad")
        t2_pad = dram.tile([32, NSAMP, H + 2, WD], F32, tag="t2_pad")
        t3_pad = dram.tile([96, NSAMP, H + 2, WD], F32, tag="t3_pad")
        t4_pad = dram.tile([32, NSAMP, H + 2, WD], F32, tag="t4_pad")
        y_buf = dram.tile([128, NSAMP, H, WD], F32, tag="y_buf")

        # zero the pad rows once
        zt = consts.tile([128, WD], F32, tag="zeros")
        nc.vector.memset(zt, 0.0)
        eps_t = consts.tile([128, 1], F32, tag="eps")
        nc.vector.memset(eps_t, EPS)
        for buf, c in ((x_pad, 64), (t_pad, 64), (t2_pad, 32), (t3_pad, 96), (t4_pad, 32)):
            for n in range(NSAMP):
                nc.sync.dma_start(out=buf[:, n, 0, :], in_=zt[0:c, :])
                nc.sync.dma_start(out=buf[:, n, H + 1, :], in_=zt[0:c, :])

        def bn_finalize(name, c, st):
            """Aggregate bn_stats groups, allreduce (mean, var, mean^2)
            pairwise, return global mean and rsd."""
            mv = stats.tile([c, 2], F32, tag=f"{name}_mv")
            nc.vector.bn_aggr(out=mv, in_=st)
            cc3 = stats.tile([c, 3], F32, tag=f"{name}_cc3")
            nc.vector.tensor_copy(out=cc3[:, 0:2], in_=mv)
            nc.vector.tensor_mul(cc3[:, 2:3], mv[:, 0:1], mv[:, 0:1])
            cci = dram.tile([c, 3], F32, tag=f"{name}_cci")
            cco = dram.tile([c, 3], F32, tag=f"{name}_cco")
            nc.sync.dma_start(out=cci, in_=cc3)
            nc.gpsimd.collective_compute(
                "AllReduce", ADD, replica_groups=PAIRS, ins=[cci.opt()], outs=[cco.opt()])
            g = stats.tile([c, 3], F32, tag=f"{name}_g")
            nc.sync.dma_start(out=g, in_=cco)
            mean = stats.tile([c, 1], F32, tag=f"{name}_mean")
            nc.scalar.mul(mean, g[:, 0:1], 0.5)
            msq = stats.tile([c, 1], F32, tag=f"{name}_msq")
            nc.vector.tensor_mul(msq, mean, mean)
            vv = stats.tile([c, 1], F32, tag=f"{name}_vv")
            nc.vector.tensor_add(vv, g[:, 1:2], g[:, 2:3])
            var = stats.tile([c, 1], F32, tag=f"{name}_var")
            nc.vector.tensor_scalar(out=var, in0=vv, scalar1=0.5,
                                    scalar2=msq, op0=MULT, op1=SUB)
            sd = stats.tile([c, 1], F32, tag=f"{name}_sd")
            nc.scalar.activation(out=sd, in_=var, func=AX.Sqrt, bias=eps_t[0:c], scale=1.0)
            rsd = stats.tile([c, 1], F32, tag=f"{name}_rsd")
            nc.vector.reciprocal(out=rsd, in_=sd)
            return mean, rsd

        def conv_pass(name, srcs, mean_ap, lhsT_taps, lhsT_1x1, bias_ap, c_out,
                      out_dram, out_padded, relu_out, want_stats, first_pad_fix):
            """Generic conv layer over all (n, macro) tiles.

            srcs: list of (dram_buf, part0, csrc, src_padded) loaded into one
            raw tile.  mean_ap: [Ctot,1] -> apply relu(v-mean) before matmul.
            lhsT_taps: [Cin, 9, Cout] for 3x3 (else None + lhsT_1x1 [Cin,Cout]).
            """
            is3 = lhsT_taps is not None
            rows_in = RM + 2 if is3 else RM
            ctot = sum(s[2] for s in srcs)
            st = None
            if want_stats:
                st = stats.tile([c_out, NSAMP * NMACRO * NSUB, 6], F32, tag=f"{name}_st")
            with tc.tile_pool(name=f"{name}_sb", bufs=3) as sb, \
                 tc.tile_pool(name=f"{name}_ps", bufs=4, space="PSUM") as psp:
                for n in range(NSAMP):
                    for mi in range(NMACRO):
                        r0 = mi * RM
                        raw = sb.tile([ctot, rows_in, WD], F32, tag=f"{name}_raw")
                        for (buf, p0, csrc, src_padded) in srcs:
                            assert src_padded
                            rs = r0 if is3 else r0 + 1
                            nc.sync.dma_start(
                                out=raw[p0:p0 + csrc],
                                in_=buf[:, n, rs:rs + rows_in, :])
                        if mean_ap is not None:
                            rhs = sb.tile([ctot, rows_in, WD], F32, tag=f"{name}_rhs")
                            nc.vector.tensor_scalar(
                                out=rhs, in0=raw, scalar1=mean_ap, scalar2=0.0,
                                op0=SUB, op1=MAX)
                            if first_pad_fix and is3:
                                # pad rows hold raw zeros; relu(0-mean) != 0,
                                # so re-zero the normalized pad row
                                if mi == 0:
                                    nc.vector.memset(rhs[:, 0, :], 0.0)
                                if mi == NMACRO - 1:
                                    nc.vector.memset(rhs[:, rows_in - 1, :], 0.0)
                        else:
                            rhs = raw
                        om = sb.tile([c_out, RM, WD], F32, tag=f"{name}_om")
                        for s in range(NSUB):
                            ps = psp.tile([c_out, 2, WD], F32, tag=f"{name}_psb")
                            if is3:
                                for i, (dy, dx) in enumerate(TAPS9_MM):
                                    t = (dy + 1) * 3 + (dx + 1)
                                    ls = 2 * s + 1 + dy
                                    if dx == 0:
                                        rap, pap = rhs[:, ls:ls + 2, :], ps[:, :, :]
                                    elif dx == -1:
                                        rap, pap = rhs[:, ls:ls + 2, 0:WD - 1], ps[:, :, 1:WD]
                                    else:
                                        rap, pap = rhs[:, ls:ls + 2, 1:WD], ps[:, :, 0:WD - 1]
                                    nc.tensor.matmul(pap, lhsT_taps[:, t, :], rap,
                                                     start=(i == 0), stop=(i == 8))
                            else:
                                nc.tensor.matmul(ps, lhsT_1x1, rhs[:, 2 * s:2 * s + 2, :],
                                                 start=True, stop=True)
                            oslice = om[:, 2 * s:2 * s + 2, :]
                            idx = (n * NMACRO + mi) * NSUB + s
                            nc.scalar.activation(
                                out=oslice, in_=ps,
                                func=AX.Relu if relu_out else AX.Identity,
                                bias=bias_ap, scale=1.0)
                            if want_stats:
                                nc.vector.bn_stats(
                                    out=st[:, idx, :],
                                    in_=oslice.rearrange("c r x -> c (r x)"))
                        po = r0 + (1 if out_padded else 0)
                        nc.sync.dma_start(out=out_dram[:, n, po:po + RM, :], in_=om)
            return st

        # ---- P1: pre conv (wg 4ch -> x 64ch, 3x3 via stacked-tap K=36) ----
        with tc.tile_pool(name="p1_sb", bufs=3) as sb, \
             tc.tile_pool(name="p1_ps", bufs=4, space="PSUM") as psp:
            st_x = stats.tile([64, NSAMP * NMACRO * NSUB, 6], F32, tag="x_st")
            for n in range(NSAMP):
                for mi in range(NMACRO):
                    r0 = mi * RM
                    stk = sb.tile([36, RM, WD], F32, tag="p1_stk")
                    for t, (dy, dx) in enumerate(TAPS9):
                        nc.sync.dma_start(
                            out=stk[4 * t:4 * t + 4],
                            in_=wg_t[:, n, r0 + 1 + dy:r0 + 1 + dy + RM, 1 + dx:1 + dx + WD])
                    om = sb.tile([64, RM, WD], F32, tag="p1_om")
                    for s in range(NSUB):
                        ps = psp.tile([64, 2, WD], F32, tag="p1_psb")
                        nc.tensor.matmul(ps, w["pre_w"], stk[:, 2 * s:2 * s + 2, :],
                                         start=True, stop=True)
                        oslice = om[:, 2 * s:2 * s + 2, :]
                        idx = (n * NMACRO + mi) * NSUB + s
                        nc.scalar.activation(out=oslice, in_=ps, func=AX.Identity,
                                             bias=w["pre_b"], scale=1.0)
                        nc.vector.bn_stats(
                            out=st_x[:, idx, :],
                            in_=oslice.rearrange("c r x -> c (r x)"))
                    nc.sync.dma_start(out=x_pad[:, n, r0 + 1:r0 + 1 + RM, :], in_=om)

        mean_x, rsd_x = bn_finalize("bnx", 64, st_x)
        nc.vector.tensor_scalar_mul(w["d1a_w"], w["d1a_w"], rsd_x)

        # ---- P2: d1a 1x1 (xn 64 -> t 64) ----
        st_t = conv_pass(
            "p2", [(x_pad, 0, 64, True)], mean_x, None, w["d1a_w"], w["d1a_b"],
            64, t_pad, True, False, True, False)
        mean_t, rsd_t = bn_finalize("bnt", 64, st_t)
        nc.vector.tensor_scalar_mul(w["d1b_w"], w["d1b_w"], rsd_t)

        # ---- P3: d1b 3x3 (tn 64 -> t2 32) ----
        st_t2 = conv_pass(
            "p3", [(t_pad, 0, 64, True)], mean_t, w["d1b_w"], None, w["d1b_b"],
            32, t2_pad, True, False, True, True)
        mean_t2, rsd_t2 = bn_finalize("bnt2", 32, st_t2)

        m96 = stats.tile([96, 1], F32, tag="m96")
        r96 = stats.tile([96, 1], F32, tag="r96")
        nc.sync.dma_start(out=m96[0:64], in_=mean_x)
        nc.sync.dma_start(out=m96[64:96], in_=mean_t2)
        nc.sync.dma_start(out=r96[0:64], in_=rsd_x)
        nc.sync.dma_start(out=r96[64:96], in_=rsd_t2)
        nc.vector.tensor_scalar_mul(w["d2a_w"], w["d2a_w"], r96)

        # ---- P4: d2a 1x1 (concat96n -> t3 96) ----
        st_t3 = conv_pass(
            "p4", [(x_pad, 0, 64, True), (t2_pad, 64, 32, True)], m96, None,
            w["d2a_w"], w["d2a_b"], 96, t3_pad, True, False, True, False)
        mean_t3, rsd_t3 = bn_finalize("bnt3", 96, st_t3)
        nc.vector.tensor_scalar_mul(w["d2b_w"], w["d2b_w"], rsd_t3)

        # ---- P5: d2b 3x3 (t3n 96 -> t4 32) ----
        st_t4 = conv_pass(
            "p5", [(t3_pad, 0, 96, True)], mean_t3, w["d2b_w"], None, w["d2b_b"],
            32, t4_pad, True, False, True, True)
        mean_t4, rsd_t4 = bn_finalize("bnt4", 32, st_t4)

        m128 = stats.tile([128, 1], F32, tag="m128")
        r128 = stats.tile([128, 1], F32, tag="r128")
        nc.sync.dma_start(out=m128[0:64], in_=mean_x)
        nc.sync.dma_start(out=m128[64:96], in_=mean_t2)
        nc.sync.dma_start(out=m128[96:128], in_=mean_t4)
        nc.sync.dma_start(out=r128[0:64], in_=rsd_x)
        nc.sync.dma_start(out=r128[64:96], in_=rsd_t2)
        nc.sync.dma_start(out=r128[96:128], in_=rsd_t4)
        nc.vector.tensor_scalar_mul(w["main_w"], w["main_w"], r128)

        # ---- P6: main 3x3 (concat128n -> y 128, relu) ----
        conv_pass(
            "p6", [(x_pad, 0, 64, True), (t2_pad, 64, 32, True), (t4_pad, 96, 32, True)],
            m128, w["main_w"], None, w["main_b"], 128, y_buf, False, True, False, True)

        # ---- P7: heads + softmax + dynamic filter + pixel shuffle ----
        # out is written [n, py, R, px, C]; host interleaves to [n, 2R+py, 2C+px]
        with tc.tile_pool(name="p7_sb", bufs=3) as sb, \
             tc.tile_pool(name="p7_ps", bufs=1, space="PSUM") as psp:
            for n in range(NSAMP):
                for mi in range(NMACRO):
                    r0 = mi * RM
                    yt = sb.tile([128, RM, WD], F32, tag="p7_y")
                    nc.sync.dma_start(out=yt, in_=y_buf[:, n, r0:r0 + RM, :])
                    pat = sb.tile([25, RM, WD], F32, tag="p7_pat")
                    # per dy-group: dims (dx, r, c) match dst (k=5g+dx, r, c)
                    base = img_t[n, :, :]
                    for g in range(5):
                        ap_src = bass.AP(
                            tensor=base.tensor,
                            offset=base.offset + (r0 + g) * (WD + 4),
                            ap=[[1, 5], [WD + 4, RM], [1, WD]])
                        nc.sync.dma_start(out=pat[5 * g:5 * g + 5], in_=ap_src)
                    om = sb.tile([4, RM, WD], F32, tag="p7_om")
                    for s in range(NSUB):
                        rhs = yt[:, 2 * s:2 * s + 2, :]
                        ps_r1 = psp.tile([128, 2, WD], F32, tag="ps_r1")
                        nc.tensor.matmul(ps_r1, w["r1_w"], rhs, start=True, stop=True)
                        ps_f1a = psp.tile([128, 2, WD], F32, tag="ps_f1a")
                        nc.tensor.matmul(ps_f1a, w["f1_w"][:, 0:128], rhs, start=True, stop=True)
                        ps_f1b = psp.tile([128, 2, WD], F32, tag="ps_f1b")
                        nc.tensor.matmul(ps_f1b, w["f1_w"][:, 128:256], rhs, start=True, stop=True)
                        rr = sb.tile([128, 2, WD], F32, tag="p7_rr")
                        nc.scalar.activation(out=rr, in_=ps_r1, func=AX.Relu,
                                             bias=w["r1_b"], scale=1.0)
                        ff0 = sb.tile([128, 2, WD], F32, tag="p7_ff0")
                        nc.scalar.activation(out=ff0, in_=ps_f1a, func=AX.Relu,
                                             bias=w["f1_b"][:, 0:1], scale=1.0)
                        ff1 = sb.tile([128, 2, WD], F32, tag="p7_ff1")
                        nc.scalar.activation(out=ff1, in_=ps_f1b, func=AX.Relu,
                                             bias=w["f1_b"][:, 1:2], scale=1.0)
                        ps_r2 = psp.tile([4, 2, WD], F32, tag="ps_r2")
                        nc.tensor.matmul(ps_r2, w["r2_w"], rr, start=True, stop=True)
                        radd = sb.tile([4, 2, WD], F32, tag="p7_radd")
                        nc.scalar.activation(out=radd, in_=ps_r2, func=AX.Identity,
                                             bias=w["r2_b"], scale=1.0)
                        ps_f2 = psp.tile([100, 2, WD], F32, tag="ps_f2")
                        nc.tensor.matmul(ps_f2, w["f2_w"][:, 0, :], ff0, start=True, stop=False)
                        nc.tensor.matmul(ps_f2, w["f2_w"][:, 1, :], ff1, start=False, stop=True)
                        e = sb.tile([100, 2, WD], F32, tag="p7_e")
                        nc.scalar.activation(out=e, in_=ps_f2, func=AX.Exp,
                                             bias=w["f2_b"], scale=1.0)
                        ps_rep = psp.tile([100, 2, WD], F32, tag="ps_rep")
                        nc.tensor.matmul(ps_rep, w["rmat"], pat[:, 2 * s:2 * s + 2, :],
                                         start=True, stop=True)
                        prod = sb.tile([100, 2, WD], F32, tag="p7_prod")
                        nc.vector.tensor_mul(prod, e, ps_rep)
                        ps_s = psp.tile([4, 2, WD], F32, tag="ps_s")
                        nc.tensor.matmul(ps_s, w["bmat"], e, start=True, stop=True)
                        ps_t = psp.tile([4, 2, WD], F32, tag="ps_t")
                        nc.tensor.matmul(ps_t, w["bmat"], prod, start=True, stop=True)
                        rec = sb.tile([4, 2, WD], F32, tag="p7_rec")
                        nc.vector.reciprocal(out=rec, in_=ps_s)
                        td = sb.tile([4, 2, WD], F32, tag="p7_td")
                        nc.vector.tensor_mul(td, ps_t, rec)
                        nc.vector.tensor_add(om[:, 2 * s:2 * s + 2, :], td, radd)
                    # pixel shuffle: split per py to keep DMA APs at 3 dims
                    for py in range(2):
                        dst = out_t[n, py, r0:r0 + RM, :, :].rearrange("r px c -> px r c")
                        nc.sync.dma_start(out=dst, in_=om[2 * py:2 * py + 2])

    nsplit = _split_multi_waits(nc)
    print(f"split {nsplit} waits onto NoOps", file=sys.stderr)
    return nc


# ---------------------------------------------------------------------------
# host side
# ---------------------------------------------------------------------------


def _pack_block_weights(p):
    def lt(a):  # [O, C, 1, 1] -> [C, O]
        return np.ascontiguousarray(np.asarray(a, np.float32)[:, :, 0, 0].T)

    def lt3(a):  # [O, C, 3, 3] -> [C, 9, O]
        a = np.asarray(a, np.float32)
        return np.ascontiguousarray(a.transpose(2, 3, 1, 0).reshape(9, a.shape[1], a.shape[0]).transpose(1, 0, 2))

    def col(b):
        return np.asarray(b, np.float32).reshape(-1, 1)

    pre = np.asarray(p["pre_w"], np.float32)  # [64, 4, 3, 3]
    pre_w = np.ascontiguousarray(pre.transpose(2, 3, 1, 0).reshape(36, 64))

    perm = np.array([(c % 25) * 4 + (c // 25) for c in range(100)])
    f2 = np.asarray(p["f2_w"], np.float32)[:, :, 0, 0]  # [100, 256]
    f2_lhsT = f2[perm, :].T  # [256, 100]
    f2_w = np.ascontiguousarray(f2_lhsT.reshape(2, 128, 100).transpose(1, 0, 2))
    f2_b = col(np.asarray(p["f2_b"], np.float32)[perm])

    bmat = np.zeros((100, 4), np.float32)
    rmat = np.zeros((25, 100), np.float32)
    for r in range(4):
        for k in range(25):
            bmat[r * 25 + k, r] = 1.0
            rmat[k, r * 25 + k] = 1.0

    f1_b = np.ascontiguousarray(np.asarray(p["f1_b"], np.float32).reshape(2, 128).T)

    return {
        "pre_w": pre_w, "pre_b": col(p["pre_b"]),
        "d1a_w": lt(p["d1a_w"]), "d1a_b": col(p["d1a_b"]),
        "d1b_w": lt3(p["d1b_w"]), "d1b_b": col(p["d1b_b"]),
        "d2a_w": lt(p["d2a_w"]), "d2a_b": col(p["d2a_b"]),
        "d2b_w": lt3(p["d2b_w"]), "d2b_b": col(p["d2b_b"]),
        "main_w": lt3(p["main_w"]), "main_b": col(p["main_b"]),
        "r1_w": lt(p["r1_w"]), "r1_b": col(p["r1_b"]),
        "r2_w": lt(p["r2_w"]), "r2_b": col(p["r2_b"]),
        "f1_w": lt(p["f1_w"]), "f1_b": f1_b,
        "f2_w": f2_w, "f2_b": f2_b,
        "bmat": bmat, "rmat": rmat,
    }


def kernel(bggr, W, params):
    global _PROG, LAST_EXEC_NS
    from concourse.bass_utils import run_bass_kernel_spmd

    bggr = np.asarray(bggr, np.float32)
    Wf = np.asarray(W, np.float32)
    wg = np.stack([Wf[:, 0, 0::2, 0::2], Wf[:, 0, 0::2, 1::2],
                   Wf[:, 0, 1::2, 0::2], Wf[:, 0, 1::2, 1::2]], 1)  # [4, 4, 256, 256]

    if _PROG is None:
        _PROG = _build_program()
    nc = _PROG

    blk_w = [_pack_block_weights({k: np.asarray(v) for k, v in p.items()})
             for p in params]

    in_maps = []
    for c in range(8):
        blk, half = c // 2, c % 2
        n0 = half * 2
        wgp = np.zeros((4, NSAMP, H + 2, WD + 2), np.float32)
        wgp[:, :, 1:H + 1, 1:WD + 1] = wg[n0:n0 + 2].transpose(1, 0, 2, 3)
        img = np.zeros((NSAMP, H + 4, WD + 4), np.float32)
        img[:, 2:H + 2, 2:WD + 2] = bggr[n0:n0 + 2, blk]
        m = {"wg": wgp, "img": img}
        m.update(blk_w[blk])
        in_maps.append(m)

    if PROFILE:
        _install_ntff_hook()
    res = run_bass_kernel_spmd(nc, in_maps, list(range(8)), trace=PROFILE)
    LAST_EXEC_NS = res.exec_time_ns

    out = np.empty((4, 4, 2 * H, 2 * WD), np.float32)
    for c in range(8):
        blk, half = c // 2, c % 2
        n0 = half * 2
        o = res.results[c]["out"]  # [n, py, R, px, C]
        out[n0:n0 + 2, blk] = o.transpose(0, 2, 1, 4, 3).reshape(NSAMP, 2 * H, 2 * WD)
    return out
